# revision 1
# baseline (speedup 1.0000x reference)
"""Self-contained Trainium2 Bass kernel: fused multi-head self-attention + LayerNorm.

Problem: B=4, S=2048, D=768, H=12 (head_dim 64), fp32 reference.

Sharding (no collectives): 8 cores = (batch b, query-half hf).  Each core
computes K/V for its full batch sequence (S=2048) and the attention output,
projection, residual and LayerNorm for its own 1024 query rows.  The context
is host-rolled per core so the core's query rows are always rows 0:1024
(softmax/attention are permutation-invariant over the key axis).

fp8 strategy: all big matmuls (QKV, V, attnV, proj) run fp8e4m3 with
perf_mode=DoubleRow (2 contraction rows per cycle).  Host pre-scales the
weights so fp8 dynamic range is used (wq,wk x16; wv x4; wp x32); the exp
folds the descale (1/(16*16*8) = 1/2048) into its scale argument and the
projection epilogue descales by 1/2048.  attnT carries attn*64 (ones column
of V = 4/64 so 1/den does the x64).  Scores stay bf16 (K=64 row-tiled
pairs).  Residual + LayerNorm in fp32.  Simulated rel err ~1.3e-4.

Device pipeline per core:
  1. x [2048,768] -> SBUF (bf16), PE-transpose -> xT [768,2048] fp8
  2. QKV: DoubleRow fp8 matmuls over din pairs; qT/kT bf16, V fp8 (JIT per
     key tile, +ones column = softmax denominator)
  3. per head pair: scores^T = kT.T@qT (bf16 row-tiled), exp on ACT with
     mask bias and 1/2048 scale -> P^T fp8; attnV DoubleRow over kt pairs
     accumulated in PSUM; normalization via broadcast reciprocal.
  4. proj (DoubleRow fp8) descaled + b_proj + residual + LayerNorm (fp32),
     DMA out.
"""

import os
from contextlib import ExitStack

import numpy as np

import concourse.bass as bass
import concourse.bacc as bacc
import concourse.tile as tile
from concourse import mybir
from concourse.masks import make_identity
from concourse.bass_utils import run_bass_kernel_spmd

# ---- problem constants (hardcoded per harness contract) ----
B, S, D, H = 4, 2048, 768, 12
HD = 64
P = 128
NCORES = 8
SQ = S // 2  # query rows per core

F32 = mybir.dt.float32
BF16 = mybir.dt.bfloat16
F8 = mybir.dt.float8e4
AF = mybir.ActivationFunctionType
ALU = mybir.AluOpType
DR = mybir.MatmulPerfMode.DoubleRow
EPS = 1e-5

# fp8 scaling scheme
S_QK = 16.0        # wq, wk host scale
S_V = 4.0          # wv host scale
S_P = 32.0         # wp host scale
ONES_C = S_V / 64.0        # V ones-column value -> attnT = attn_true * 64
EXP_SCALE = 1.0 / (S_QK * S_QK * 8.0)   # descale scores + 1/sqrt(HD)
PROJ_DESCALE = 1.0 / (64.0 * S_P)

# DVE fast-exp bit trick: fp8e4m3 bits of exp(a) ~= round(8*log2(e)*a + B8);
# computed as one tensor_scalar (int8 out) straight off the scores PSUM.
A8 = 8.0 * np.log2(np.e) * EXP_SCALE        # applied to raw (scaled) scores
B8 = 8.0 * (7.0 - 0.04303)                  # exp bias + minimax correction
A8M = 8.0 * np.log2(np.e)                   # applied to the mask bias

TRACE = bool(int(os.environ.get("KERNEL_TRACE", "0")))
LAST_RESULTS = None


def declare_io(nc, S_, SQ_, D_):
    io = {}
    io["ctx"] = nc.dram_tensor("ctx", [S_, D_], F32, kind="ExternalInput")
    io["wq"] = nc.dram_tensor("wq", [D_, D_], F8, kind="ExternalInput")
    io["wk"] = nc.dram_tensor("wk", [D_, D_], F8, kind="ExternalInput")
    io["wv"] = nc.dram_tensor("wv", [D_, D_], F8, kind="ExternalInput")
    io["wp"] = nc.dram_tensor("wp", [D_, D_], F8, kind="ExternalInput")
    io["bproj"] = nc.dram_tensor("bproj", [D_], F32, kind="ExternalInput")
    io["gamma"] = nc.dram_tensor("gamma", [D_], F32, kind="ExternalInput")
    io["beta"] = nc.dram_tensor("beta", [D_], F32, kind="ExternalInput")
    io["mbias"] = nc.dram_tensor("mbias", [P, S_ // P], F32, kind="ExternalInput")
    io["mb8"] = nc.dram_tensor("mb8", [P, S_ // P], F32, kind="ExternalInput")
    io["out"] = nc.dram_tensor("out", [SQ_, D_], F32, kind="ExternalOutput")
    return io


def emit_mhsa(ctx: ExitStack, tc: tile.TileContext, io, S_, SQ_, D_, H_):
    nc = tc.nc
    KT = S_ // P        # key-token tiles
    DT = D_ // P        # feature tiles (also head pairs)
    DP = DT // 2        # feature-tile pairs (DoubleRow contraction steps)
    PAIRS = H_ // 2
    assert PAIRS == DT
    QB = min(512, SQ_)  # query block (matmul N)
    NQB = SQ_ // QB
    QTT = SQ_ // P      # query token tiles (proj phase)
    HE = HD + 1         # V columns per head incl. ones column (denominator)
    HEP = 80            # padded V head stride (keeps kt-pair stride 16B-aligned)

    ctx_r = io["ctx"][:, :].rearrange("(t p) d -> p t d", p=P)
    out_r = io["out"][:, :].rearrange("(t p) d -> p t d", p=P)

    # ---------------- constants ----------------
    const = ctx.enter_context(tc.tile_pool(name="const", bufs=1))
    ident = const.tile([P, P], BF16)
    make_identity(nc, ident)
    mb = const.tile([P, KT], F32)
    nc.gpsimd.dma_start(out=mb, in_=io["mbias"][:, :])
    mb8 = const.tile([P, KT], F32)
    nc.gpsimd.dma_start(out=mb8, in_=io["mb8"][:, :])
    eps_t = const.tile([P, 1], F32)
    nc.vector.memset(eps_t, EPS)
    bproj_bc = const.tile([P, D_], F32)
    gamma_bc = const.tile([P, D_], F32)
    beta_bc = const.tile([P, D_], F32)

    # ---------------- persistent big tiles ----------------
    big = ctx.enter_context(tc.tile_pool(name="big", bufs=1))
    qT = big.tile([P, DT, SQ_], BF16, tag="qT")      # [pair-feats, pair, qtok]
    kTt = big.tile([P, DT, S_], BF16, tag="kT")      # [pair-feats, pair, ktok]
    V = big.tile([P, KT, H_ * HEP], F8, tag="V")     # [tok, ktile, head-major +ones]
    attnT = big.tile([P, DT, SQ_], F8, tag="attnT")  # attn_true * 64

    wpool = ctx.enter_context(tc.tile_pool(name="wpool", bufs=1))
    wq_sb = wpool.tile([P, DT, D_], F8, tag="wq_sb")
    wpsb = wpool.tile([P, DT, D_], F8, tag="wpsb")
    wk_sb = wpool.tile([P, DT, D_], F8, tag="wk_sb")
    wv_sb = wpool.tile([P, DT, D_], F8, tag="wv_sb")
    with tc.tile_pool(name="xt_pool", bufs=1) as xt_pool:
        xT = xt_pool.tile([P, DT, S_], F8, tag="xT")

        # ---------------- phase 1: load (casting) + PE transpose ------------
        # transpose via matmul against identity (x tile stationary): runs at
        # regular-matmul rate (HAM-warmable) instead of transpose-mode; copies
        # out of PSUM are batched 4 token-tiles at a time (fp32 -> fp8 cast).
        with tc.tile_pool(name="xn_pool", bufs=2) as xn_pool, \
             tc.tile_pool(name="tr_ps", bufs=2, space="PSUM") as tr_ps:
            TB = 4
            for tq in range(KT // TB):
                xbs = []
                for i in range(TB):
                    xb = xn_pool.tile([P, D_], BF16, tag=f"xb{i}",
                                      name=f"xb_{tq}_{i}")
                    nc.gpsimd.dma_start(out=xb, in_=ctx_r[:, tq * TB + i, :])
                    xbs.append(xb)
                for d in range(DT):
                    ps = tr_ps.tile([P, TB * P], F32, tag="trps")
                    for i in range(TB):
                        nc.tensor.matmul(
                            ps[:, i * P:(i + 1) * P],
                            lhsT=xbs[i][:, d * P:(d + 1) * P],
                            rhs=ident, start=True, stop=True)
                    nc.scalar.copy(
                        out=xT[:, d, tq * TB * P:(tq + 1) * TB * P], in_=ps)

        # weight loads + broadcast constants AFTER the x loads so they don't
        # delay the transposes at the head of the pipeline
        for din in range(DT):
            nc.sync.dma_start(out=wq_sb[:, din, :],
                              in_=io["wq"][din * P:(din + 1) * P, :])
            nc.sync.dma_start(out=wk_sb[:, din, :],
                              in_=io["wk"][din * P:(din + 1) * P, :])
            nc.sync.dma_start(out=wv_sb[:, din, :],
                              in_=io["wv"][din * P:(din + 1) * P, :])
        for din in range(DT):
            nc.sync.dma_start(out=wpsb[:, din, :],
                              in_=io["wp"][din * P:(din + 1) * P, :])
        nc.sync.dma_start(out=bproj_bc, in_=io["bproj"][:].partition_broadcast(P))
        nc.sync.dma_start(out=gamma_bc, in_=io["gamma"][:].partition_broadcast(P))
        nc.sync.dma_start(out=beta_bc, in_=io["beta"][:].partition_broadcast(P))

        V_h = V.rearrange("p t (h e) -> p t h e", e=HEP)

        # pools for the projection/LayerNorm epilogue (emitted inside the
        # attention block so the last tiles overlap the final pairs)
        epi = ExitStack()
        res_pool = epi.enter_context(tc.tile_pool(name="res_pool", bufs=2))
        y_pool = epi.enter_context(tc.tile_pool(name="y_pool", bufs=4))
        st_pool = epi.enter_context(tc.tile_pool(name="st_pool", bufs=8))

        def emit_proj_tiles(qkv_ps, t_lo, t_hi):
            # prefetch all residual tiles up front (ring DMA, off-engine)
            x_ress = {}
            for t in range(t_lo, t_hi):
                x_res = res_pool.tile([P, D_], F32, tag=f"xres{t % 8}",
                                      name=f"xres_{t}")
                nc.sync.dma_start(out=x_res, in_=ctx_r[:, t, :])
                x_ress[t] = x_res
            for t in range(t_lo, t_hi):
                x_res = x_ress[t]
                y = y_pool.tile([P, D_], F32, tag="y", name=f"y_{t}")
                for c0, cw in ((0, 512), (512, 256)):
                    pp = qkv_ps.tile([P, QB], F32, tag="av",
                                     name=f"pp_{t}_{c0}")
                    for dp in range(DP):
                        nc.tensor.matmul(
                            pp[:, 0:cw],
                            lhsT=attnT[:, 2 * dp:2 * dp + 2, t * P:(t + 1) * P],
                            rhs=wpsb[:, 2 * dp:2 * dp + 2, c0:c0 + cw],
                            start=(dp == 0),
                            stop=(dp == DP - 1),
                            perf_mode=DR,
                        )
                    # y = pp * PROJ_DESCALE + x_res (fused fp8 descale)
                    nc.vector.scalar_tensor_tensor(
                        out=y[:, c0:c0 + cw], in0=pp[:, 0:cw],
                        scalar=PROJ_DESCALE, in1=x_res[:, c0:c0 + cw],
                        op0=ALU.mult, op1=ALU.add)
                nc.vector.tensor_add(out=y, in0=y, in1=bproj_bc)
                # LayerNorm over D
                nsub = D_ // 256
                stats = st_pool.tile([P, nsub, nc.vector.BN_STATS_DIM], F32,
                                     tag="stats", name=f"stats_{t}")
                for g in range(nsub):
                    nc.vector.bn_stats(out=stats[:, g, :],
                                       in_=y[:, g * 256:(g + 1) * 256])
                mv = st_pool.tile([P, 2], F32, tag="mv", name=f"mv_{t}")
                nc.vector.bn_aggr(out=mv, in_=stats)
                sd = st_pool.tile([P, 1], F32, tag="sd", name=f"sd_{t}")
                nc.scalar.activation(
                    out=sd, in_=mv[:, 1:2], func=AF.Sqrt, bias=eps_t, scale=1.0)
                nc.vector.reciprocal(out=sd, in_=sd)
                # fused LN apply: ((y - mu) * gamma) * rstd + beta
                t1 = y_pool.tile([P, D_], F32, tag="t1", name=f"t1_{t}", bufs=2)
                nc.vector.scalar_tensor_tensor(
                    out=t1, in0=y, scalar=mv[:, 0:1], in1=gamma_bc,
                    op0=ALU.subtract, op1=ALU.mult)
                yf = y_pool.tile([P, D_], F32, tag="yf", name=f"yf_{t}", bufs=2)
                nc.vector.scalar_tensor_tensor(
                    out=yf, in0=t1, scalar=sd, in1=beta_bc,
                    op0=ALU.mult, op1=ALU.add)
                nc.sync.dma_start(out=out_r[:, t, :], in_=yf)

        # ---------------- phase 3: per-pair QKV + attention ----------------
        # PSUM budget: s_ps 3x2 banks + av ring 2 banks = 8.  The av ring is
        # tag-shared by the V-phase / QK-projection / proj psum tiles (all
        # [P,512] fp32), which only run while the av accumulators are dead.
        with tc.tile_pool(name="s_ps", bufs=3, space="PSUM") as s_ps, \
             tc.tile_pool(name="av_ps", bufs=2, space="PSUM") as av_ps, \
             tc.tile_pool(name="pt_pool", bufs=4) as pt_pool, \
             tc.tile_pool(name="dr_pool", bufs=2, space="DRAM") as dr_pool, \
             tc.tile_pool(name="r_pool", bufs=2) as r_pool:
            qkv_ps = av_ps  # alias: V/QK/proj psums share the av ring

            # ---------- phase 2: all of V (dedicated, before attention) -----
            for kt in range(KT):
                for c0, cw in ((0, 512), (512, 256)):
                    pv = av_ps.tile([P, QB], F32, tag="av",
                                    name=f"pv_{kt}_{c0}")
                    for dp in range(DP):
                        nc.tensor.matmul(
                            pv[:, 0:cw],
                            lhsT=xT[:, 2 * dp:2 * dp + 2, kt * P:(kt + 1) * P],
                            rhs=wv_sb[:, 2 * dp:2 * dp + 2, c0:c0 + cw],
                            start=(dp == 0),
                            stop=(dp == DP - 1),
                            perf_mode=DR,
                        )
                    nc.scalar.copy(
                        out=V_h[:, kt, c0 // HD:(c0 + cw) // HD, 0:HD],
                        in_=pv[:, 0:cw].rearrange("p (h e) -> p h e", e=HD))
                nc.gpsimd.memset(V_h[:, kt, :, HD:HE], ONES_C)

            for p in range(PAIRS):
                # qT / kT projections for this head pair (m-tile == p).
                # Block order k0,q0,k1,k2,k3,q1: the first scores of the
                # attention loop only need k-block 0 + q-block 0, so the PE
                # can enter the loop after two copies instead of six.
                blocks = [(wk_sb, kTt, 0), (wq_sb, qT, 0), (wk_sb, kTt, 1),
                          (wk_sb, kTt, 2), (wk_sb, kTt, 3)]
                if NQB > 1:
                    blocks.append((wq_sb, qT, 1))
                nblk = 512
                for bi, (wsb, dst, nb) in enumerate(blocks):
                    pq = av_ps.tile([P, QB], F32, tag="av",
                                    name=f"pq_{p}_{bi}")
                    for dp in range(DP):
                        nc.tensor.matmul(
                            pq[:, 0:nblk],
                            lhsT=wsb[:, 2 * dp:2 * dp + 2, p * P:(p + 1) * P],
                            rhs=xT[:, 2 * dp:2 * dp + 2,
                                   nb * nblk:(nb + 1) * nblk],
                            start=(dp == 0),
                            stop=(dp == DP - 1),
                            perf_mode=DR,
                        )
                    nc.scalar.copy(
                        out=dst[:, p, nb * nblk:(nb + 1) * nblk],
                        in_=pq[:, 0:nblk])

                for qbi in range(NQB):
                    av = [av_ps.tile([P, QB], F32, tag="av", name=f"av_{p}_{qbi}_{i}")
                          for i in range(2)]

                    # attnV runs LAG ktp-steps behind the scores/exp stage so
                    # the PE never waits on the exp engines (software pipeline)
                    LAG = 2
                    pt2s = {}

                    def emit_attnv(ktp):
                        pt2 = pt2s.pop(ktp)
                        for hh in range(2):
                            h = 2 * p + hh
                            nc.tensor.matmul(
                                av[hh][0:HE, :],
                                lhsT=V_h[:, 2 * ktp:2 * ktp + 2, h, 0:HE],
                                rhs=pt2[:, :, hh * QB:(hh + 1) * QB],
                                start=(ktp == 0), stop=(ktp == KT // 2 - 1),
                                perf_mode=DR,
                            )

                    for ktp in range(KT // 2):
                        pt2 = pt_pool.tile([P, 2, 2 * QB], F8, tag="pt",
                                           name=f"pt_{p}_{qbi}_{ktp}")
                        pt2s[ktp] = pt2
                        for j in range(2):
                            kt = 2 * ktp + j
                            sh = s_ps.tile([P, 2 * QB], F32, tag="sh")
                            # transposed scores, head pair row-tiled on the PE
                            nc.tensor.matmul(
                                sh[:, 0:QB],
                                lhsT=kTt[0:HD, p, kt * P:(kt + 1) * P],
                                rhs=qT[0:HD, p, qbi * QB:(qbi + 1) * QB],
                                start=True, stop=True, tile_position=(0, 0),
                            )
                            nc.tensor.matmul(
                                sh[:, QB:2 * QB],
                                lhsT=kTt[HD:P, p, kt * P:(kt + 1) * P],
                                rhs=qT[HD:P, p, qbi * QB:(qbi + 1) * QB],
                                start=True, stop=True, tile_position=(64, 0),
                            )
                            # exp(scores/2048 + mask_bias[ktok]) -> P^T (fp8).
                            # Split across engines: ACT (table-exact) for 60%,
                            # DVE bit-trick (int8 = fp8e4m3 bits of exp) for
                            # 40%, so neither stalls the PE inner loop.
                            if j == 0 and ktp % 8 != 7:
                                nc.vector.tensor_scalar(
                                    out=pt2[:, j, :].bitcast(mybir.dt.int8),
                                    in0=sh, scalar1=float(A8),
                                    scalar2=mb8[:, kt:kt + 1],
                                    op0=ALU.mult, op1=ALU.add)
                            else:
                                nc.scalar.activation(
                                    out=pt2[:, j, :], in_=sh, func=AF.Exp,
                                    bias=mb[:, kt:kt + 1], scale=EXP_SCALE)
                        # attnV DoubleRow (lagged); ones row = denominator
                        if ktp >= LAG:
                            emit_attnv(ktp - LAG)
                    for ktp in range(KT // 2 - LAG, KT // 2):
                        emit_attnv(ktp)
                    # Drain unnormalized numerators + reciprocal rows right
                    # away so the PSUM banks free for the next block; the
                    # normalization (DRAM-bounced partition broadcast of 1/den)
                    # completes asynchronously off the PE critical path.
                    den_sb = r_pool.tile([P, QB], F32, tag="den_sb")
                    den_dr = dr_pool.tile([2, QB], F32, tag="den_dr")
                    R = r_pool.tile([HD, 2, QB], F32, tag="R")
                    u = r_pool.tile([P, 2, QB], F32, tag="u")
                    ub = r_pool.tile([HD, QB], F8, tag="ub")
                    den_cp = r_pool.tile([P, QB], F32, tag="den_cp")
                    for hh in range(2):
                        # copy numerators + denom row out of PSUM first (frees
                        # the bank); the custom-DVE reciprocal needs a
                        # partition-0 fp32 SBUF source, so stage the denom row
                        nc.scalar.copy(
                            out=u[0:HD, hh, :], in_=av[hh][0:HD, :])
                        nc.vector.tensor_copy(
                            out=den_cp[0:1, :], in_=av[hh][HD:HE, :])
                        nc.vector.reciprocal_approx_fast(
                            out=den_sb[0:1, :], in_=den_cp[0:1, :])
                        nc.gpsimd.dma_start(
                            out=den_dr[hh:hh + 1, :], in_=den_sb[0:1, :])
                        nc.gpsimd.dma_start(
                            out=R[:, hh, :],
                            in_=den_dr[hh, :].partition_broadcast(HD))
                    cols = slice(qbi * QB, (qbi + 1) * QB)
                    nc.vector.tensor_mul(
                        out=attnT[0:HD, p, cols], in0=u[0:HD, 0, :], in1=R[:, 0, :])
                    nc.vector.tensor_mul(
                        out=ub, in0=u[0:HD, 1, :], in1=R[:, 1, :])
                    nc.gpsimd.dma_start(
                        out=attnT[HD:P, p, cols], in_=ub)

                    if p == PAIRS - 1:
                        half = (QTT // NQB) * (qbi + 1) if NQB > 1 else QTT
                        lo = (QTT // NQB) * qbi if NQB > 1 else 0
                        if qbi == NQB - 1:
                            half = QTT
                        emit_proj_tiles(qkv_ps, lo, half)


        epi.close()


def build_program(S_=S, SQ_=SQ, D_=D, H_=H):
    nc = bacc.Bacc("TRN2")
    io = declare_io(nc, S_, SQ_, D_)
    with tile.TileContext(nc) as tc:
        with ExitStack() as ctx:
            emit_mhsa(ctx, tc, io, S_, SQ_, D_, H_)
    nc.compile()
    return nc, io


def prep_inputs(context, pad_mask, w_qkv, w_proj, b_proj, gamma, beta,
                S_=S, SQ_=SQ, D_=D, ncores=NCORES):
    import ml_dtypes
    f8 = ml_dtypes.float8_e4m3fn
    context = np.asarray(context, dtype=np.float32)
    pad_mask = np.asarray(pad_mask, dtype=np.float32)
    w_qkv = np.asarray(w_qkv, dtype=np.float32)
    wq = (np.ascontiguousarray(w_qkv[:, 0:D_]) * np.float32(S_QK)).astype(f8)
    wk = (np.ascontiguousarray(w_qkv[:, D_:2 * D_]) * np.float32(S_QK)).astype(f8)
    wv = (np.ascontiguousarray(w_qkv[:, 2 * D_:3 * D_]) * np.float32(S_V)).astype(f8)
    wp = (np.asarray(w_proj, dtype=np.float32) * np.float32(S_P)).astype(f8)
    bp = np.asarray(b_proj, dtype=np.float32)
    ga = np.asarray(gamma, dtype=np.float32)
    be = np.asarray(beta, dtype=np.float32)
    mbias = (pad_mask - 1.0) * np.float32(1e10)  # [B, S]
    in_maps = []
    for c in range(ncores):
        b, hf = c // 2, c % 2
        ctx_c = np.ascontiguousarray(np.roll(context[b], -hf * SQ_, axis=0))
        mb_c = np.ascontiguousarray(
            np.roll(mbias[b], -hf * SQ_).reshape(S_ // P, P).T).astype(np.float32)
        mb8_c = (mb_c * np.float32(A8M) + np.float32(B8)).astype(np.float32)
        in_maps.append({
            "ctx": ctx_c, "wq": wq, "wk": wk, "wv": wv, "wp": wp,
            "bproj": bp, "gamma": ga, "beta": be, "mbias": mb_c,
            "mb8": mb8_c,
        })
    return in_maps


def kernel(context, pad_mask, w_qkv, w_proj, b_proj, gamma, beta):
    global LAST_RESULTS
    nc, _io = build_program()
    in_maps = prep_inputs(context, pad_mask, w_qkv, w_proj, b_proj, gamma, beta)
    res = run_bass_kernel_spmd(nc, in_maps, core_ids=list(range(NCORES)),
                               trace=TRACE)
    LAST_RESULTS = res
    out = np.empty((B, S, D), np.float32)
    for c in range(NCORES):
        b, hf = c // 2, c % 2
        out[b, hf * SQ:(hf + 1) * SQ] = res.results[c]["out"]
    return out

